# revision 8
# baseline (speedup 1.0000x reference)
"""Autoformer-style EncoderLayer (series-decomp + single-head attention + FFN)
for Trainium2, data-parallel over batch across 8 NeuronCores.

Per core: one [L=2048, D=512] sequence.
  trend = AvgPool1d(x, k=25, pad=12, count_include_pad=True)
  s     = x - trend                        (banded matmul: S = B @ x, B = I - A)
  Q,K,V = s@wq+bq, s@wk, s@wv              (bk cancels in softmax; bv folds into bo)
  attn  = softmax(Q K^T / sqrt(D))         (computed transposed: scores^T[m,l])
  h     = LN1(s + attn@V@wo + bo')         (bo' = bo + bv@wo)
  out   = LN2(h + relu(h@w1+bb1)@w2+bb2) + trend

Matmul operand dtypes: banded seasonal in f32r; scores (Q.K), attn@V and the
softmax denominator in fp8-e4m3 with DoubleRow perf mode (2x PE rate,
256-deep contraction per pass); everything else in bf16 (same PE rate as f32r
at N=512 but half the LDWEIGHTS cost and half the SBUF). All PSUM accumulation
and LayerNorm statistics stay f32. exp() is computed with a -ln(8) bias so u
fits fp8's +-240 range; the softmax normalization cancels the factor.

Everything is SBUF-resident (s / trend / h / Q^T / K^T / V / u) -- no DRAM
spills. Phase C work (LN1 + h^T + FFN + LN2) for block lb is emitted between
attention blocks lb+1 and lb+2 so the PE never sees a phase-transition bubble
(which previously also re-triggered the HAM half-rate throttle window).
"""
import math
import numpy as np
import ml_dtypes
from contextlib import ExitStack

import concourse.bass as bass
import concourse.mybir as mybir
import concourse.tile as tile
from concourse import bacc
from concourse.bass_utils import run_bass_kernel_spmd

P = 128
B_, L, D = 8, 2048, 512
KPOOL, PAD = 25, 12
EPS = 1e-5
SCALE = 1.0 / math.sqrt(D)
LOG8 = math.log(8.0)
NLC = L // P          # 16 l-chunks of 128
NB = L // 512         # 4  l-blocks of 512
ND = D // P           # 4  d-chunks of 128
USE_FP8 = True

f32 = mybir.dt.float32
f32r = mybir.dt.float32r
bf16 = mybir.dt.bfloat16
fp8 = mybir.dt.float8e4
AF = mybir.ActivationFunctionType
ALU = mybir.AluOpType
DRM = mybir.MatmulPerfMode.DoubleRow

_CACHE = {}


def _band_blocks():
    i = np.arange(P)[:, None]
    j = np.arange(P)[None, :]
    a = (np.abs(i - j) <= PAD).astype(np.float32) / KPOOL
    bdiag = np.eye(P, dtype=np.float32) - a
    bup = -((i - j) >= (P - PAD)).astype(np.float32) / KPOOL   # rows chunk c-1, cols chunk c
    bdown = bup.T.copy()                                       # rows chunk c+1, cols chunk c
    return bdiag, bup, bdown


def _ln_block(nc, small, t_sum, t_ssq, t_eps):
    """Per-block LayerNorm stats on [P, 4]: returns (istd, nmi, negmean)."""
    t_mean = small.tile([P, 4], f32, tag="lbm", name="tb_mean")
    nc.vector.tensor_scalar_mul(t_mean[:], t_sum[:], 1.0 / D)
    t_m2 = small.tile([P, 4], f32, tag="lbm2", name="tb_m2")
    nc.vector.tensor_tensor(t_m2[:], t_mean[:], t_mean[:], ALU.mult)
    t_var = small.tile([P, 4], f32, tag="lbv", name="tb_var")
    nc.vector.scalar_tensor_tensor(t_var[:], t_ssq[:], 1.0 / D, t_m2[:],
                                   op0=ALU.mult, op1=ALU.subtract)
    t_sd = small.tile([P, 4], f32, tag="lbsd", name="tb_sd")
    nc.scalar.activation(t_sd[:], t_var[:], AF.Sqrt, bias=t_eps[:])
    t_istd = small.tile([P, 4], f32, tag="lbi", name="tb_istd")
    nc.vector.reciprocal(t_istd[:], t_sd[:])
    t_nmi = small.tile([P, 4], f32, tag="lbn", name="tb_nmi")
    nc.vector.scalar_tensor_tensor(t_nmi[:], t_mean[:], -1.0, t_istd[:],
                                   op0=ALU.mult, op1=ALU.mult)
    t_negm = small.tile([P, 4], f32, tag="lbng", name="tb_negm")
    nc.vector.tensor_scalar_mul(t_negm[:], t_mean[:], -1.0)
    return t_istd, t_nmi, t_negm


def _build(apply_g1, apply_g2):
    nc = bacc.Bacc("TRN2", target_bir_lowering=False, debug=False)

    def din(name, shape, dt):
        return nc.dram_tensor(name, list(shape), dt, kind="ExternalInput").ap()

    x = din("x", (L, D), f32)
    ws = {n: din(n, (D, D), bf16) for n in ["wq", "wk", "wv", "wo", "w1", "w2"]}
    cpk = din("cpk", (P, 400), f32)
    cbf = din("cbf", (P, 129), bf16)
    rbf = din("rbf", (1, 1154), bf16)
    gb = {}
    if apply_g1:
        gb["g1b"] = din("g1b", (P, D), bf16)
        gb["be1b"] = din("be1b", (P, D), bf16)
    if apply_g2:
        gb["g2b"] = din("g2b", (P, D), bf16)
        gb["be2b"] = din("be2b", (P, D), bf16)

    out = nc.dram_tensor("out", [L, D], f32, kind="ExternalOutput").ap()
    out_c = out.rearrange("(l p) d -> l p d", p=P)

    adt = fp8 if USE_FP8 else bf16   # attention operand dtype (q/k/v/u)

    with tile.TileContext(nc) as tc, ExitStack() as ctx:
        misc = ctx.enter_context(tc.tile_pool(name="misc", bufs=1))
        small = ctx.enter_context(tc.tile_pool(name="small", bufs=4))
        ps_mm = ctx.enter_context(tc.tile_pool(name="ps_mm", bufs=4, space="PSUM"))
        ps_tr = ctx.enter_context(tc.tile_pool(name="ps_tr", bufs=1, space="PSUM"))
        ps_den = ctx.enter_context(tc.tile_pool(name="ps_den", bufs=1, space="PSUM"))

        # ---- constants ----
        t_cpk = misc.tile([P, 400], f32r, name="t_cpk")
        nc.sync.dma_start(t_cpk[:], cpk.bitcast(f32r))
        t_cbf = misc.tile([P, 129], bf16, name="t_cbf")
        nc.sync.dma_start(t_cbf[:], cbf)
        t_rbf = misc.tile([1, 1154], bf16, name="t_rbf")
        nc.sync.dma_start(t_rbf[:], rbf)
        t_bd = t_cpk[:, 0:128]
        t_bu = t_cpk[:, 128:256]
        t_bn = t_cpk[:, 256:384]
        t_eps = t_cpk[:, 384:385].bitcast(f32)
        t_bq = t_cpk[:, 385:389].bitcast(f32)
        t_bb1 = t_cpk[:, 389:393].bitcast(f32)
        t_o2 = t_cpk[0:1, 393:395]                    # f32r ones [1,2]
        t_nl8 = t_cpk[:, 395:396].bitcast(f32)        # -ln(8)
        t_id = t_cbf[:, 0:128]                        # bf16 identity
        t_ocol = t_cbf[:, 128:129]                    # bf16 ones column
        r_bo2 = t_rbf[:, 0:512]
        r_bb2 = t_rbf[:, 512:1024]
        r_ones = t_rbf[:, 1024:1152]
        if USE_FP8:
            t_ones8 = misc.tile([P, 2, 16], fp8, name="t_ones8")
            nc.any.memset(t_ones8[:], 1.0)
        t_gb = {}
        for n in gb:
            t_gb[n] = misc.tile([P, D], bf16, name=f"t_{n}")
            nc.sync.dma_start(t_gb[n][:], gb[n][:])

        # ---- SBUF residents ----
        s_res = misc.tile([P, NLC, D], bf16, name="s_res")
        tr_res = misc.tile([P, NLC, D], bf16, name="tr_res")
        h_res = misc.tile([P, NLC, D], bf16, name="h_res")
        t_sum1a = misc.tile([P, NLC], f32, name="t_sum1a")
        t_ssq1a = misc.tile([P, NLC], f32, name="t_ssq1a")

        wop = ctx.enter_context(tc.tile_pool(name="wop", bufs=1))
        t_wo = wop.tile([P, ND, D], bf16, name="t_wo")
        t_w1 = wop.tile([P, ND, D], bf16, name="t_w1")
        t_w2 = wop.tile([P, ND, D], bf16, name="t_w2")

        # attention operand tensors
        abp = ctx.enter_context(tc.tile_pool(name="abp", bufs=1))
        t_qt = abp.tile([P, ND, L], adt, name="t_qt")
        t_kt = abp.tile([P, ND, L], adt, name="t_kt")
        t_v = abp.tile([P, NLC, D], adt, name="t_v")
        t_u = abp.tile([P, NLC, 512], adt, name="t_u")

        # ---- phase A: x -> s, trend, S^T, Q^T, K^T, V ----
        es_a = ExitStack()
        apool = es_a.enter_context(tc.tile_pool(name="apool", bufs=1))
        xwin = es_a.enter_context(tc.tile_pool(name="xwin", bufs=16))
        wqkv = es_a.enter_context(tc.tile_pool(name="wqkv", bufs=1))
        stb = apool.tile([P, ND, L], bf16, name="stb")
        x_cview = x.rearrange("(l p) d -> p l d", p=P)
        x_ch = []
        for j in range(NLC):
            t = xwin.tile([P, D], f32r, tag="xw", name=f"xw{j}")
            nc.sync.dma_start(t[:], x_cview[:, j, :].bitcast(f32r))
            x_ch.append(t)
        t_w = {}
        for n in ["wq", "wk", "wv"]:
            t_w[n] = wqkv.tile([P, ND, D], bf16, name=f"t_w_{n}")
            nc.sync.dma_start(
                t_w[n][:], ws[n].rearrange("(k p) n -> p k n", p=P))
        # later-phase weights: DMA now, queue drains behind x + wqkv
        nc.sync.dma_start(t_wo[:], ws["wo"].rearrange("(k p) n -> p k n", p=P))
        nc.sync.dma_start(t_w1[:], ws["w1"].rearrange("(k p) n -> p k n", p=P))
        nc.sync.dma_start(t_w2[:], ws["w2"].rearrange("(k p) n -> p k n", p=P))

        # banded seasonal + trend for all 16 chunks
        for lc in range(NLC):
            pss = ps_mm.tile([P, D], f32, tag="mm", name="pss")
            nbrs = [(lc - 1, t_bu), (lc, t_bd), (lc + 1, t_bn)]
            nbrs = [(j, t) for j, t in nbrs if 0 <= j < NLC]
            for i, (j, tb) in enumerate(nbrs):
                nc.tensor.matmul(pss[:], tb[:], x_ch[j][:],
                                 start=(i == 0), stop=(i == len(nbrs) - 1))
            nc.scalar.copy(s_res[:, lc, :], pss[:])
            nc.gpsimd.tensor_tensor(tr_res[:, lc, :], x_ch[lc][:].bitcast(f32),
                                    s_res[:, lc, :], ALU.subtract)

        # per block: S^T, then Q^T/K^T, then V
        for lb in range(NB):
            for c in range(4):
                lc = lb * 4 + c
                for dc in range(ND):
                    ptt = ps_tr.tile([P, P], f32, tag="pt", bufs=2, name="ptt")
                    nc.tensor.matmul(ptt[:], s_res[:, lc, bass.ts(dc, P)],
                                     t_id[:], start=True, stop=True)
                    nc.scalar.copy(stb[:, dc, bass.ts(lc, P)], ptt[:])
            for tdst, wname, has_b in [(t_qt, "wq", True), (t_kt, "wk", False)]:
                for dc in range(ND):
                    pq = ps_mm.tile([P, 512], f32, tag="mm", name="pq")
                    for k in range(ND):
                        nc.tensor.matmul(pq[:], t_w[wname][:, k, bass.ts(dc, P)],
                                         stb[:, k, bass.ts(lb, 512)],
                                         start=(k == 0), stop=(k == ND - 1))
                    if has_b:
                        nc.scalar.activation(tdst[:, dc, bass.ts(lb, 512)], pq[:],
                                             AF.Identity, bias=t_bq[:, dc:dc + 1])
                    else:
                        nc.scalar.copy(tdst[:, dc, bass.ts(lb, 512)], pq[:])
            for c in range(4):
                lc = lb * 4 + c
                pv = ps_mm.tile([P, D], f32, tag="mm", name="pv")
                for k in range(ND):
                    nc.tensor.matmul(pv[:], stb[:, k, bass.ts(lc, P)],
                                     t_w["wv"][:, k, :],
                                     start=(k == 0), stop=(k == ND - 1))
                nc.scalar.copy(t_v[:, lc, :], pv[:])
        es_a.close()   # frees stb, x window, wq/wk/wv

        # ---- phase B/C pools ----
        bcp = ctx.enter_context(tc.tile_pool(name="bcp", bufs=1))
        cpool = ctx.enter_context(tc.tile_pool(name="cpool", bufs=1))
        t_ht = cpool.tile([P, ND, L], bf16, name="t_ht")
        t_rt = cpool.tile([P, ND, L], bf16, name="t_rt")

        def emit_B(lb):
            # scores^T (u[m, l]) with in-loop denominator accumulation
            pden = ps_den.tile([1, 512], f32, tag="den", name="pden")
            u_bf = []
            for mc in range(NLC):
                psc = ps_mm.tile([P, 512], f32, tag="mm", name="psc")
                if USE_FP8:
                    for k2 in (0, 2):
                        nc.tensor.matmul(psc[:], t_kt[:, k2:k2 + 2, bass.ts(mc, P)],
                                         t_qt[:, k2:k2 + 2, bass.ts(lb, 512)],
                                         start=(k2 == 0), stop=(k2 == 2),
                                         perf_mode=DRM)
                    nc.scalar.activation(t_u[:, mc, :], psc[:], AF.Exp,
                                         scale=SCALE, bias=t_nl8[:])
                    if mc % 2 == 1:
                        nc.tensor.matmul(pden[:], t_ones8[:, :, 0:1],
                                         t_u[:, mc - 1:mc + 1, :],
                                         start=(mc == 1), stop=(mc == NLC - 1),
                                         perf_mode=DRM)
                else:
                    for k in range(ND):
                        nc.tensor.matmul(psc[:], t_kt[:, k, bass.ts(mc, P)],
                                         t_qt[:, k, bass.ts(lb, 512)],
                                         start=(k == 0), stop=(k == ND - 1))
                    nc.scalar.activation(t_u[:, mc, :], psc[:], AF.Exp,
                                         scale=SCALE)
                    nc.tensor.matmul(pden[:], t_ocol[:], t_u[:, mc, :],
                                     start=(mc == 0), stop=(mc == NLC - 1))
            den_bf = small.tile([1, 512], bf16, tag="denb", name="den_bf")
            nc.scalar.copy(den_bf[:], pden[:])
            den_f = small.tile([1, 512], f32r, tag="denf", name="den_f")
            nc.scalar.copy(den_f[:], pden[:])
            prc = ps_tr.tile([P, 4, 2], f32, tag="rec", name="prc")
            for c in range(4):
                nc.tensor.matmul(prc[:, c, :], den_f[:, bass.ts(c, P)],
                                 t_o2[:], start=True, stop=True)
            t_rec = small.tile([P, 4], f32, tag="recs", name="t_rec")
            nc.vector.reciprocal(t_rec[:], prc[:, :, 0])

            # attn @ V  (transposed: avt[d, l])
            t_avt = bcp.tile([P, ND, 512], bf16, tag="avt", bufs=2, name="t_avt")
            for dc in range(ND):
                pav = ps_mm.tile([P, 512], f32, tag="mm", name="pav")
                if USE_FP8:
                    for m2 in range(0, NLC, 2):
                        nc.tensor.matmul(pav[:], t_v[:, m2:m2 + 2, bass.ts(dc, P)],
                                         t_u[:, m2:m2 + 2, :],
                                         start=(m2 == 0), stop=(m2 == NLC - 2),
                                         perf_mode=DRM)
                else:
                    for mc in range(NLC):
                        nc.tensor.matmul(pav[:], t_v[:, mc, bass.ts(dc, P)],
                                         t_u[:, mc, :],
                                         start=(mc == 0), stop=(mc == NLC - 1))
                nc.vector.tensor_copy(t_avt[:, dc, :], pav[:])

            # wo projection back to natural [l, d] + residual + LN1 stats
            rs_slab = bcp.tile([P, 4, D], f32, tag="rs", bufs=2, name="rs_slab")
            for c in range(4):
                lc = lb * 4 + c
                pwo = ps_mm.tile([P, D], f32, tag="mm", name="pwo")
                for k in range(ND):
                    nc.tensor.matmul(pwo[:], t_avt[:, k, bass.ts(c, P)],
                                     t_wo[:, k, :],
                                     start=(k == 0), stop=False)
                nc.tensor.matmul(pwo[:], den_bf[:, bass.ts(c, P)],
                                 r_bo2[:], start=False, stop=True)
                nc.vector.scalar_tensor_tensor(
                    rs_slab[:, c, :], pwo[:], t_rec[:, c:c + 1],
                    s_res[:, lc, :],
                    op0=ALU.mult, op1=ALU.add,
                    accum_out=t_sum1a[:, lc:lc + 1])
                t_scr = bcp.tile([P, D], f32, tag="sqscr", bufs=2, name="t_scr")
                nc.scalar.activation(t_scr[:], rs_slab[:, c, :], AF.Square,
                                     accum_out=t_ssq1a[:, lc:lc + 1])
            return rs_slab

        def emit_C(lb, rs_slab):
            # LN1 normalize -> h (bf16), h^T, ff1+relu, ff2, LN2, +trend, out
            t_istd4, t_nmi4, t_negm4 = _ln_block(
                nc, small, t_sum1a[:, lb * 4:lb * 4 + 4],
                t_ssq1a[:, lb * 4:lb * 4 + 4], t_eps)
            for c in range(4):
                lc = lb * 4 + c
                nc.vector.tensor_scalar(h_res[:, lc, :], rs_slab[:, c, :],
                                        t_negm4[:, c:c + 1], t_istd4[:, c:c + 1],
                                        op0=ALU.add, op1=ALU.mult)
                if apply_g1:
                    nc.vector.tensor_tensor(h_res[:, lc, :], h_res[:, lc, :],
                                            t_gb["g1b"][:], ALU.mult)
                    nc.vector.tensor_tensor(h_res[:, lc, :], h_res[:, lc, :],
                                            t_gb["be1b"][:], ALU.add)
            for c in range(4):
                lc = lb * 4 + c
                for dc in range(ND):
                    pht = ps_tr.tile([P, P], f32, tag="pt", bufs=2, name="pht")
                    nc.tensor.matmul(pht[:], h_res[:, lc, bass.ts(dc, P)],
                                     t_id[:], start=True, stop=True)
                    nc.scalar.copy(t_ht[:, dc, bass.ts(lc, P)], pht[:])
            for dc in range(ND):
                pf = ps_mm.tile([P, 512], f32, tag="mm", name="pf")
                for k in range(ND):
                    nc.tensor.matmul(pf[:], t_w1[:, k, bass.ts(dc, P)],
                                     t_ht[:, k, bass.ts(lb, 512)],
                                     start=(k == 0), stop=(k == ND - 1))
                nc.scalar.activation(t_rt[:, dc, bass.ts(lb, 512)], pf[:],
                                     AF.Relu, bias=t_bb1[:, dc:dc + 1])
            t_sum2b = small.tile([P, 4], f32, tag="sum2b", name="t_sum2b")
            t_ssq2b = small.tile([P, 4], f32, tag="ssq2b", name="t_ssq2b")
            res_list = []
            for c in range(4):
                lc = lb * 4 + c
                pf2 = ps_mm.tile([P, D], f32, tag="mm", name="pf2")
                for k in range(ND):
                    nc.tensor.matmul(pf2[:], t_rt[:, k, bass.ts(lc, P)],
                                     t_w2[:, k, :],
                                     start=(k == 0), stop=False)
                nc.tensor.matmul(pf2[:], r_ones[:], r_bb2[:],
                                 start=False, stop=True)
                t_res = cpool.tile([P, D], f32, tag="res2", bufs=5, name="t_res2")
                nc.vector.scalar_tensor_tensor(
                    t_res[:], pf2[:], 1.0, h_res[:, lc, :],
                    op0=ALU.mult, op1=ALU.add,
                    accum_out=t_sum2b[:, c:c + 1])
                t_scr = cpool.tile([P, D], f32, tag="sqscr2", bufs=2,
                                   name="t_scr2")
                nc.scalar.activation(t_scr[:], t_res[:], AF.Square,
                                     accum_out=t_ssq2b[:, c:c + 1])
                res_list.append(t_res)
            t_istd4, t_nmi4, t_negm4 = _ln_block(
                nc, small, t_sum2b, t_ssq2b, t_eps)
            for c in range(4):
                lc = lb * 4 + c
                t_h2 = cpool.tile([P, D], f32, tag="h2out", bufs=4, name="t_h2")
                if c % 2 == 1:
                    nc.scalar.activation(t_h2[:], res_list[c][:], AF.Identity,
                                         scale=t_istd4[:, c:c + 1],
                                         bias=t_nmi4[:, c:c + 1])
                else:
                    nc.vector.tensor_scalar(t_h2[:], res_list[c][:],
                                            t_negm4[:, c:c + 1],
                                            t_istd4[:, c:c + 1],
                                            op0=ALU.add, op1=ALU.mult)
                if apply_g2:
                    nc.vector.tensor_tensor(t_h2[:], t_h2[:],
                                            t_gb["g2b"][:], ALU.mult)
                    nc.vector.tensor_tensor(t_h2[:], t_h2[:],
                                            t_gb["be2b"][:], ALU.add)
                t_out = cpool.tile([P, D], f32, tag="outst", bufs=4, name="t_out")
                eng = nc.gpsimd if c % 2 == 0 else nc.vector
                eng.tensor_tensor(t_out[:], t_h2[:], tr_res[:, lc, :], ALU.add)
                nc.gpsimd.dma_start(out_c[lc], t_out[:])

        # B0, B1, C0, B2, C1, B3, C2, C3
        rs = {}
        rs[0] = emit_B(0)
        rs[1] = emit_B(1)
        emit_C(0, rs[0])
        rs[2] = emit_B(2)
        emit_C(1, rs[1])
        rs[3] = emit_B(3)
        emit_C(2, rs[2])
        emit_C(3, rs[3])

    nc.compile()
    return nc


def _consts(inp):
    bdiag, bup, bdown = _band_blocks()
    cpk = np.zeros((P, 400), np.float32)
    cpk[:, 0:128] = bdiag
    cpk[:, 128:256] = bup
    cpk[:, 256:384] = bdown
    cpk[:, 384] = EPS
    cpk[:, 385:389] = inp["bq"].reshape(ND, P).T
    cpk[:, 389:393] = inp["bb1"].reshape(ND, P).T
    cpk[:, 393:395] = 1.0
    cpk[:, 395] = -LOG8
    cbf = np.zeros((P, 129), ml_dtypes.bfloat16)
    cbf[:, 0:128] = np.eye(P, dtype=np.float32)
    cbf[:, 128] = 1.0
    wo_b = inp["wo"].astype(ml_dtypes.bfloat16).astype(np.float32)
    bo2 = inp["bo"] + inp["bv"].astype(ml_dtypes.bfloat16).astype(np.float32) @ wo_b
    rbf = np.zeros((1, 1154), ml_dtypes.bfloat16)
    rbf[0, 0:512] = bo2
    rbf[0, 512:1024] = inp["bb2"]
    rbf[0, 1024:1152] = 1.0
    consts = {"cpk": cpk, "cbf": cbf, "rbf": rbf}
    for n in ["wq", "wk", "wv", "wo", "w1", "w2"]:
        consts[n] = inp[n].astype(ml_dtypes.bfloat16)
    return consts


def _prepare(inputs):
    inp = {k: np.ascontiguousarray(np.asarray(v, dtype=np.float32))
           for k, v in inputs.items()}
    x = inp["x"]                      # [8, 2048, 512]
    assert x.shape == (B_, L, D)

    apply_g1 = not (np.allclose(inp["g1"], 1.0) and np.allclose(inp["be1"], 0.0))
    apply_g2 = not (np.allclose(inp["g2"], 1.0) and np.allclose(inp["be2"], 0.0))

    key = (apply_g1, apply_g2)
    if key not in _CACHE:
        _CACHE[key] = _build(apply_g1, apply_g2)
    nc = _CACHE[key]

    consts = _consts(inp)
    if apply_g1:
        consts["g1b"] = np.tile(inp["g1"].reshape(1, D), (P, 1)).astype(ml_dtypes.bfloat16)
        consts["be1b"] = np.tile(inp["be1"].reshape(1, D), (P, 1)).astype(ml_dtypes.bfloat16)
    if apply_g2:
        consts["g2b"] = np.tile(inp["g2"].reshape(1, D), (P, 1)).astype(ml_dtypes.bfloat16)
        consts["be2b"] = np.tile(inp["be2"].reshape(1, D), (P, 1)).astype(ml_dtypes.bfloat16)
    consts = {k: np.ascontiguousarray(v) for k, v in consts.items()}
    in_maps = [dict(consts, x=np.ascontiguousarray(x[i])) for i in range(B_)]
    return nc, in_maps


def kernel(**inputs):
    nc, in_maps = _prepare(inputs)
    res = run_bass_kernel_spmd(nc, in_maps, core_ids=list(range(B_)))
    return np.stack([res.results[i]["out"] for i in range(B_)], axis=0)


# revision 10
# speedup vs baseline: 1.0956x; 1.0956x over previous
"""Autoformer-style EncoderLayer (series-decomp + single-head attention + FFN)
for Trainium2, data-parallel over batch across 8 NeuronCores.

Per core: one [L=2048, D=512] sequence.
  trend = AvgPool1d(x, k=25, pad=12, count_include_pad=True)
  s     = x - trend                        (banded matmul: S = B @ x, B = I - A)
  Q,K,V = s@wq+bq, s@wk, s@wv              (bk cancels in softmax; bv folds into bo)
  attn  = softmax(Q K^T / sqrt(D))         (computed transposed: scores^T[m,l])
  h     = LN1(s + attn@V@wo + bo')         (bo' = bo + bv@wo)
  out   = LN2(h + relu(h@w1+bb1)@w2+bb2) + trend

Matmul operand dtypes: banded seasonal in f32r; scores (Q.K), attn@V and the
softmax denominator in fp8-e4m3 with DoubleRow perf mode (2x PE rate,
256-deep contraction per pass); everything else in bf16 (same PE rate as f32r
at N=512 but half the LDWEIGHTS cost and half the SBUF). All PSUM accumulation
and LayerNorm statistics stay f32. exp() is computed with a -ln(8) bias so u
fits fp8's +-240 range; the softmax normalization cancels the factor.

Everything is SBUF-resident (s / trend / h / Q^T / K^T / V / u) -- no DRAM
spills. Phase C work (LN1 + h^T + FFN + LN2) for block lb is emitted between
attention blocks lb+1 and lb+2 so the PE never sees a phase-transition bubble
(which previously also re-triggered the HAM half-rate throttle window).
"""
import math
import numpy as np
import ml_dtypes
from contextlib import ExitStack

import concourse.bass as bass
import concourse.mybir as mybir
import concourse.tile as tile
from concourse import bacc
from concourse.bass_utils import run_bass_kernel_spmd

P = 128
B_, L, D = 8, 2048, 512
KPOOL, PAD = 25, 12
EPS = 1e-5
SCALE = 1.0 / math.sqrt(D)
LOG8 = math.log(8.0)
NLC = L // P          # 16 l-chunks of 128
NB = L // 512         # 4  l-blocks of 512
ND = D // P           # 4  d-chunks of 128
USE_FP8 = True

f32 = mybir.dt.float32
f32r = mybir.dt.float32r
bf16 = mybir.dt.bfloat16
fp8 = mybir.dt.float8e4
AF = mybir.ActivationFunctionType
ALU = mybir.AluOpType
DRM = mybir.MatmulPerfMode.DoubleRow

_CACHE = {}


def _band_blocks():
    i = np.arange(P)[:, None]
    j = np.arange(P)[None, :]
    a = (np.abs(i - j) <= PAD).astype(np.float32) / KPOOL
    bdiag = np.eye(P, dtype=np.float32) - a
    bup = -((i - j) >= (P - PAD)).astype(np.float32) / KPOOL   # rows chunk c-1, cols chunk c
    bdown = bup.T.copy()                                       # rows chunk c+1, cols chunk c
    return bdiag, bup, bdown


def _ln_block(nc, small, t_sum, t_ssq, t_eps):
    """Per-block LayerNorm stats on [P, 4]: returns (istd, nmi, negmean)."""
    t_mean = small.tile([P, 4], f32, tag="lbm", name="tb_mean")
    nc.vector.tensor_scalar_mul(t_mean[:], t_sum[:], 1.0 / D)
    t_m2 = small.tile([P, 4], f32, tag="lbm2", name="tb_m2")
    nc.vector.tensor_tensor(t_m2[:], t_mean[:], t_mean[:], ALU.mult)
    t_var = small.tile([P, 4], f32, tag="lbv", name="tb_var")
    nc.vector.scalar_tensor_tensor(t_var[:], t_ssq[:], 1.0 / D, t_m2[:],
                                   op0=ALU.mult, op1=ALU.subtract)
    t_sd = small.tile([P, 4], f32, tag="lbsd", name="tb_sd")
    nc.scalar.activation(t_sd[:], t_var[:], AF.Sqrt, bias=t_eps[:])
    t_istd = small.tile([P, 4], f32, tag="lbi", name="tb_istd")
    nc.vector.reciprocal(t_istd[:], t_sd[:])
    t_nmi = small.tile([P, 4], f32, tag="lbn", name="tb_nmi")
    nc.vector.scalar_tensor_tensor(t_nmi[:], t_mean[:], -1.0, t_istd[:],
                                   op0=ALU.mult, op1=ALU.mult)
    t_negm = small.tile([P, 4], f32, tag="lbng", name="tb_negm")
    nc.vector.tensor_scalar_mul(t_negm[:], t_mean[:], -1.0)
    return t_istd, t_nmi, t_negm


def _build(apply_g1, apply_g2):
    nc = bacc.Bacc("TRN2", target_bir_lowering=False, debug=False)

    def din(name, shape, dt):
        return nc.dram_tensor(name, list(shape), dt, kind="ExternalInput").ap()

    x = din("x", (L, D), f32)
    ws = {n: din(n, (D, D), bf16) for n in ["wq", "wk", "wv", "wo", "w1", "w2"]}
    cpk = din("cpk", (P, 400), f32)
    cbf = din("cbf", (P, 129), bf16)
    rbf = din("rbf", (1, 1154), bf16)
    gb = {}
    if apply_g1:
        gb["g1b"] = din("g1b", (P, D), bf16)
        gb["be1b"] = din("be1b", (P, D), bf16)
    if apply_g2:
        gb["g2b"] = din("g2b", (P, D), bf16)
        gb["be2b"] = din("be2b", (P, D), bf16)

    out = nc.dram_tensor("out", [L, D], f32, kind="ExternalOutput").ap()
    out_c = out.rearrange("(l p) d -> l p d", p=P)

    adt = fp8 if USE_FP8 else bf16   # attention operand dtype (q/k/v/u)

    with tile.TileContext(nc) as tc, ExitStack() as ctx:
        misc = ctx.enter_context(tc.tile_pool(name="misc", bufs=1))
        small = ctx.enter_context(tc.tile_pool(name="small", bufs=4))
        ps_mm = ctx.enter_context(tc.tile_pool(name="ps_mm", bufs=4, space="PSUM"))
        ps_tr = ctx.enter_context(tc.tile_pool(name="ps_tr", bufs=1, space="PSUM"))
        ps_den = ctx.enter_context(tc.tile_pool(name="ps_den", bufs=1, space="PSUM"))

        # ---- constants ----
        t_cpk = misc.tile([P, 400], f32r, name="t_cpk")
        nc.sync.dma_start(t_cpk[:], cpk.bitcast(f32r))
        t_cbf = misc.tile([P, 129], bf16, name="t_cbf")
        nc.sync.dma_start(t_cbf[:], cbf)
        t_rbf = misc.tile([1, 1154], bf16, name="t_rbf")
        nc.sync.dma_start(t_rbf[:], rbf)
        t_bd = t_cpk[:, 0:128]
        t_bu = t_cpk[:, 128:256]
        t_bn = t_cpk[:, 256:384]
        t_eps = t_cpk[:, 384:385].bitcast(f32)
        t_bq = t_cpk[:, 385:389].bitcast(f32)
        t_bb1 = t_cpk[:, 389:393].bitcast(f32)
        t_o2 = t_cpk[0:1, 393:395]                    # f32r ones [1,2]
        t_nl8 = t_cpk[:, 395:396].bitcast(f32)        # -ln(8)
        t_id = t_cbf[:, 0:128]                        # bf16 identity
        t_ocol = t_cbf[:, 128:129]                    # bf16 ones column
        r_bo2 = t_rbf[:, 0:512]
        r_bb2 = t_rbf[:, 512:1024]
        r_ones = t_rbf[:, 1024:1152]
        if USE_FP8:
            t_ones8 = misc.tile([P, 2, 16], fp8, name="t_ones8")
            nc.any.memset(t_ones8[:], 1.0)
        t_gb = {}
        for n in gb:
            t_gb[n] = misc.tile([P, D], bf16, name=f"t_{n}")
            nc.sync.dma_start(t_gb[n][:], gb[n][:])

        # ---- SBUF residents ----
        s_res = misc.tile([P, NLC, D], bf16, name="s_res")
        tr_res = misc.tile([P, NLC, D], bf16, name="tr_res")
        h_res = misc.tile([P, NLC, D], bf16, name="h_res")
        t_sum1a = misc.tile([P, NLC], f32, name="t_sum1a")
        t_ssq1a = misc.tile([P, NLC], f32, name="t_ssq1a")

        wop = ctx.enter_context(tc.tile_pool(name="wop", bufs=1))
        t_wo = wop.tile([P, ND, D], bf16, name="t_wo")
        t_w1 = wop.tile([P, ND, D], bf16, name="t_w1")
        t_w2 = wop.tile([P, ND, D], bf16, name="t_w2")

        # attention operand tensors
        abp = ctx.enter_context(tc.tile_pool(name="abp", bufs=1))
        t_qt = abp.tile([P, ND, L], adt, name="t_qt")
        t_kt = abp.tile([P, ND, L], adt, name="t_kt")
        t_v = abp.tile([P, NLC, D], adt, name="t_v")
        t_u = abp.tile([P, NLC, 512], adt, name="t_u")

        # ---- phase A: x -> s, trend, S^T, Q^T, K^T, V ----
        es_a = ExitStack()
        apool = es_a.enter_context(tc.tile_pool(name="apool", bufs=1))
        xwin = es_a.enter_context(tc.tile_pool(name="xwin", bufs=16))
        wqkv = es_a.enter_context(tc.tile_pool(name="wqkv", bufs=1))
        stb = apool.tile([P, ND, L], bf16, name="stb")
        x_cview = x.rearrange("(l p) d -> p l d", p=P)
        x_ch = []
        for j in range(NLC):
            t = xwin.tile([P, D], f32r, tag="xw", name=f"xw{j}")
            nc.sync.dma_start(t[:], x_cview[:, j, :].bitcast(f32r))
            x_ch.append(t)
        # weights ride the gpsimd DMA queue, in parallel with the x stream
        t_w = {}
        for n in ["wq", "wk", "wv"]:
            t_w[n] = wqkv.tile([P, ND, D], bf16, name=f"t_w_{n}")
            nc.gpsimd.dma_start(
                t_w[n][:], ws[n].rearrange("(k p) n -> p k n", p=P))
        nc.gpsimd.dma_start(t_wo[:], ws["wo"].rearrange("(k p) n -> p k n", p=P))
        nc.gpsimd.dma_start(t_w1[:], ws["w1"].rearrange("(k p) n -> p k n", p=P))
        nc.gpsimd.dma_start(t_w2[:], ws["w2"].rearrange("(k p) n -> p k n", p=P))

        for lb in range(NB):
            # banded seasonal + trend for this block
            for c in range(4):
                lc = lb * 4 + c
                pss = ps_mm.tile([P, D], f32, tag="mm", name="pss")
                nbrs = [(lc - 1, t_bu), (lc, t_bd), (lc + 1, t_bn)]
                nbrs = [(j, t) for j, t in nbrs if 0 <= j < NLC]
                for i, (j, tb) in enumerate(nbrs):
                    nc.tensor.matmul(pss[:], tb[:], x_ch[j][:],
                                     start=(i == 0), stop=(i == len(nbrs) - 1))
                nc.scalar.copy(s_res[:, lc, :], pss[:])
                nc.gpsimd.tensor_tensor(tr_res[:, lc, :],
                                        x_ch[lc][:].bitcast(f32),
                                        s_res[:, lc, :], ALU.subtract)
            # S^T: 4 transposes per chunk into one PSUM bank, one batched drain
            for c in range(4):
                lc = lb * 4 + c
                ptt = ps_tr.tile([P, ND, P], f32, tag="pt", bufs=2, name="ptt")
                for dc in range(ND):
                    nc.tensor.matmul(ptt[:, dc, :], s_res[:, lc, bass.ts(dc, P)],
                                     t_id[:], start=True, stop=True)
                if lc % 2 == 0:
                    nc.scalar.copy(stb[:, :, bass.ts(lc, P)], ptt[:])
                else:
                    nc.vector.tensor_copy(stb[:, :, bass.ts(lc, P)], ptt[:])
            for tdst, wname, has_b in [(t_qt, "wq", True), (t_kt, "wk", False)]:
                for dc in range(ND):
                    pq = ps_mm.tile([P, 512], f32, tag="mm", name="pq")
                    for k in range(ND):
                        nc.tensor.matmul(pq[:], t_w[wname][:, k, bass.ts(dc, P)],
                                         stb[:, k, bass.ts(lb, 512)],
                                         start=(k == 0), stop=(k == ND - 1))
                    if has_b:
                        nc.scalar.activation(tdst[:, dc, bass.ts(lb, 512)], pq[:],
                                             AF.Identity, bias=t_bq[:, dc:dc + 1])
                    else:
                        nc.scalar.copy(tdst[:, dc, bass.ts(lb, 512)], pq[:])
            for c in range(4):
                lc = lb * 4 + c
                pv = ps_mm.tile([P, D], f32, tag="mm", name="pv")
                for k in range(ND):
                    nc.tensor.matmul(pv[:], stb[:, k, bass.ts(lc, P)],
                                     t_w["wv"][:, k, :],
                                     start=(k == 0), stop=(k == ND - 1))
                nc.scalar.copy(t_v[:, lc, :], pv[:])
        es_a.close()   # frees stb, x window, wq/wk/wv

        # ---- phase B/C pools ----
        bcp = ctx.enter_context(tc.tile_pool(name="bcp", bufs=1))
        cpool = ctx.enter_context(tc.tile_pool(name="cpool", bufs=1))
        t_ht = cpool.tile([P, ND, L], bf16, name="t_ht")
        t_rt = cpool.tile([P, ND, L], bf16, name="t_rt")

        def emit_B(lb):
            # scores^T (u[m, l]) with in-loop denominator accumulation
            pden = ps_den.tile([1, 512], f32, tag="den", name="pden")
            u_bf = []
            for mc in range(NLC):
                psc = ps_mm.tile([P, 512], f32, tag="mm", name="psc")
                if USE_FP8:
                    for k2 in (0, 2):
                        nc.tensor.matmul(psc[:], t_kt[:, k2:k2 + 2, bass.ts(mc, P)],
                                         t_qt[:, k2:k2 + 2, bass.ts(lb, 512)],
                                         start=(k2 == 0), stop=(k2 == 2),
                                         perf_mode=DRM)
                    nc.scalar.activation(t_u[:, mc, :], psc[:], AF.Exp,
                                         scale=SCALE, bias=t_nl8[:])
                    if mc % 2 == 1:
                        nc.tensor.matmul(pden[:], t_ones8[:, :, 0:1],
                                         t_u[:, mc - 1:mc + 1, :],
                                         start=(mc == 1), stop=(mc == NLC - 1),
                                         perf_mode=DRM)
                else:
                    for k in range(ND):
                        nc.tensor.matmul(psc[:], t_kt[:, k, bass.ts(mc, P)],
                                         t_qt[:, k, bass.ts(lb, 512)],
                                         start=(k == 0), stop=(k == ND - 1))
                    nc.scalar.activation(t_u[:, mc, :], psc[:], AF.Exp,
                                         scale=SCALE)
                    nc.tensor.matmul(pden[:], t_ocol[:], t_u[:, mc, :],
                                     start=(mc == 0), stop=(mc == NLC - 1))
            den_bf = small.tile([1, 512], bf16, tag="denb", name="den_bf")
            nc.scalar.copy(den_bf[:], pden[:])
            den_f = small.tile([1, 512], f32r, tag="denf", name="den_f")
            nc.scalar.copy(den_f[:], pden[:])
            prc = ps_tr.tile([P, 4, 2], f32, tag="rec", name="prc")
            for c in range(4):
                nc.tensor.matmul(prc[:, c, :], den_f[:, bass.ts(c, P)],
                                 t_o2[:], start=True, stop=True)
            t_rec = small.tile([P, 4], f32, tag="recs", name="t_rec")
            nc.vector.reciprocal(t_rec[:], prc[:, :, 0])

            # attn @ V  (transposed: avt[d, l])
            t_avt = bcp.tile([P, ND, 512], bf16, tag="avt", bufs=2, name="t_avt")
            for dc in range(ND):
                pav = ps_mm.tile([P, 512], f32, tag="mm", name="pav")
                if USE_FP8:
                    for m2 in range(0, NLC, 2):
                        nc.tensor.matmul(pav[:], t_v[:, m2:m2 + 2, bass.ts(dc, P)],
                                         t_u[:, m2:m2 + 2, :],
                                         start=(m2 == 0), stop=(m2 == NLC - 2),
                                         perf_mode=DRM)
                else:
                    for mc in range(NLC):
                        nc.tensor.matmul(pav[:], t_v[:, mc, bass.ts(dc, P)],
                                         t_u[:, mc, :],
                                         start=(mc == 0), stop=(mc == NLC - 1))
                nc.vector.tensor_copy(t_avt[:, dc, :], pav[:])

            # wo projection back to natural [l, d] + residual + LN1 stats
            rs_slab = bcp.tile([P, 4, D], f32, tag="rs", bufs=2, name="rs_slab")
            for c in range(4):
                lc = lb * 4 + c
                pwo = ps_mm.tile([P, D], f32, tag="mm", name="pwo")
                for k in range(ND):
                    nc.tensor.matmul(pwo[:], t_avt[:, k, bass.ts(c, P)],
                                     t_wo[:, k, :],
                                     start=(k == 0), stop=False)
                nc.tensor.matmul(pwo[:], den_bf[:, bass.ts(c, P)],
                                 r_bo2[:], start=False, stop=True)
                nc.vector.scalar_tensor_tensor(
                    rs_slab[:, c, :], pwo[:], t_rec[:, c:c + 1],
                    s_res[:, lc, :],
                    op0=ALU.mult, op1=ALU.add,
                    accum_out=t_sum1a[:, lc:lc + 1])
                t_scr = bcp.tile([P, D], f32, tag="sqscr", bufs=2, name="t_scr")
                nc.vector.scalar_tensor_tensor(
                    t_scr[:], rs_slab[:, c, :], 1.0, rs_slab[:, c, :],
                    op0=ALU.mult, op1=ALU.mult,
                    accum_out=t_ssq1a[:, lc:lc + 1])
            return rs_slab

        def emit_N(lb, rs_slab):
            # LN1 stats + normalize -> h (bf16); emitted right after emit_B(lb)
            # so the vector engine produces h while the PE runs the next
            # attention block
            t_istd4, t_nmi4, t_negm4 = _ln_block(
                nc, small, t_sum1a[:, lb * 4:lb * 4 + 4],
                t_ssq1a[:, lb * 4:lb * 4 + 4], t_eps)
            for c in range(4):
                lc = lb * 4 + c
                nc.vector.tensor_scalar(h_res[:, lc, :], rs_slab[:, c, :],
                                        t_negm4[:, c:c + 1], t_istd4[:, c:c + 1],
                                        op0=ALU.add, op1=ALU.mult)
                if apply_g1:
                    nc.vector.tensor_tensor(h_res[:, lc, :], h_res[:, lc, :],
                                            t_gb["g1b"][:], ALU.mult)
                    nc.vector.tensor_tensor(h_res[:, lc, :], h_res[:, lc, :],
                                            t_gb["be1b"][:], ALU.add)

        def emit_Cmm(lb):
            # h^T, ff1+relu, ff2, LN2, +trend, out DMA
            for c in range(4):
                lc = lb * 4 + c
                pht = ps_tr.tile([P, ND, P], f32, tag="pt", bufs=2, name="pht")
                for dc in range(ND):
                    nc.tensor.matmul(pht[:, dc, :], h_res[:, lc, bass.ts(dc, P)],
                                     t_id[:], start=True, stop=True)
                if lc % 2 == 0:
                    nc.scalar.copy(t_ht[:, :, bass.ts(lc, P)], pht[:])
                else:
                    nc.vector.tensor_copy(t_ht[:, :, bass.ts(lc, P)], pht[:])
            for dc in range(ND):
                pf = ps_mm.tile([P, 512], f32, tag="mm", name="pf")
                for k in range(ND):
                    nc.tensor.matmul(pf[:], t_w1[:, k, bass.ts(dc, P)],
                                     t_ht[:, k, bass.ts(lb, 512)],
                                     start=(k == 0), stop=(k == ND - 1))
                nc.scalar.activation(t_rt[:, dc, bass.ts(lb, 512)], pf[:],
                                     AF.Relu, bias=t_bb1[:, dc:dc + 1])
            t_sum2b = small.tile([P, 4], f32, tag="sum2b", name="t_sum2b")
            t_ssq2b = small.tile([P, 4], f32, tag="ssq2b", name="t_ssq2b")
            res_list = []
            for c in range(4):
                lc = lb * 4 + c
                pf2 = ps_mm.tile([P, D], f32, tag="mm", name="pf2")
                for k in range(ND):
                    nc.tensor.matmul(pf2[:], t_rt[:, k, bass.ts(lc, P)],
                                     t_w2[:, k, :],
                                     start=(k == 0), stop=False)
                nc.tensor.matmul(pf2[:], r_ones[:], r_bb2[:],
                                 start=False, stop=True)
                t_res = cpool.tile([P, D], f32, tag="res2", bufs=5, name="t_res2")
                nc.vector.scalar_tensor_tensor(
                    t_res[:], pf2[:], 1.0, h_res[:, lc, :],
                    op0=ALU.mult, op1=ALU.add,
                    accum_out=t_sum2b[:, c:c + 1])
                t_scr = cpool.tile([P, D], f32, tag="sqscr2", bufs=2,
                                   name="t_scr2")
                nc.scalar.activation(t_scr[:], t_res[:], AF.Square,
                                     accum_out=t_ssq2b[:, c:c + 1])
                res_list.append(t_res)
            t_istd4, t_nmi4, t_negm4 = _ln_block(
                nc, small, t_sum2b, t_ssq2b, t_eps)
            # LN2 normalize on scalar (frees the vector queue for the next
            # block's LN1), trend add split gpsimd/vector, store via gpsimd q
            for c in range(4):
                lc = lb * 4 + c
                t_h2 = cpool.tile([P, D], f32, tag="h2out", bufs=4, name="t_h2")
                nc.scalar.activation(t_h2[:], res_list[c][:], AF.Identity,
                                     scale=t_istd4[:, c:c + 1],
                                     bias=t_nmi4[:, c:c + 1])
                if apply_g2:
                    nc.vector.tensor_tensor(t_h2[:], t_h2[:],
                                            t_gb["g2b"][:], ALU.mult)
                    nc.vector.tensor_tensor(t_h2[:], t_h2[:],
                                            t_gb["be2b"][:], ALU.add)
                t_out = cpool.tile([P, D], f32, tag="outst", bufs=4, name="t_out")
                eng = nc.gpsimd if c % 2 == 0 else nc.vector
                eng.tensor_tensor(t_out[:], t_h2[:], tr_res[:, lc, :], ALU.add)
                nc.gpsimd.dma_start(out_c[lc], t_out[:])

        # B0 N0 B1 N1 C0 B2 N2 C1 B3 N3 C2 C3
        rs0 = emit_B(0); emit_N(0, rs0)
        rs1 = emit_B(1); emit_N(1, rs1)
        emit_Cmm(0)
        rs2 = emit_B(2); emit_N(2, rs2)
        emit_Cmm(1)
        rs3 = emit_B(3); emit_N(3, rs3)
        emit_Cmm(2)
        emit_Cmm(3)

    nc.compile()
    return nc


def _consts(inp):
    bdiag, bup, bdown = _band_blocks()
    cpk = np.zeros((P, 400), np.float32)
    cpk[:, 0:128] = bdiag
    cpk[:, 128:256] = bup
    cpk[:, 256:384] = bdown
    cpk[:, 384] = EPS
    cpk[:, 385:389] = inp["bq"].reshape(ND, P).T
    cpk[:, 389:393] = inp["bb1"].reshape(ND, P).T
    cpk[:, 393:395] = 1.0
    cpk[:, 395] = -LOG8
    cbf = np.zeros((P, 129), ml_dtypes.bfloat16)
    cbf[:, 0:128] = np.eye(P, dtype=np.float32)
    cbf[:, 128] = 1.0
    wo_b = inp["wo"].astype(ml_dtypes.bfloat16).astype(np.float32)
    bo2 = inp["bo"] + inp["bv"].astype(ml_dtypes.bfloat16).astype(np.float32) @ wo_b
    rbf = np.zeros((1, 1154), ml_dtypes.bfloat16)
    rbf[0, 0:512] = bo2
    rbf[0, 512:1024] = inp["bb2"]
    rbf[0, 1024:1152] = 1.0
    consts = {"cpk": cpk, "cbf": cbf, "rbf": rbf}
    for n in ["wq", "wk", "wv", "wo", "w1", "w2"]:
        consts[n] = inp[n].astype(ml_dtypes.bfloat16)
    return consts


def _prepare(inputs):
    inp = {k: np.ascontiguousarray(np.asarray(v, dtype=np.float32))
           for k, v in inputs.items()}
    x = inp["x"]                      # [8, 2048, 512]
    assert x.shape == (B_, L, D)

    apply_g1 = not (np.allclose(inp["g1"], 1.0) and np.allclose(inp["be1"], 0.0))
    apply_g2 = not (np.allclose(inp["g2"], 1.0) and np.allclose(inp["be2"], 0.0))

    key = (apply_g1, apply_g2)
    if key not in _CACHE:
        _CACHE[key] = _build(apply_g1, apply_g2)
    nc = _CACHE[key]

    consts = _consts(inp)
    if apply_g1:
        consts["g1b"] = np.tile(inp["g1"].reshape(1, D), (P, 1)).astype(ml_dtypes.bfloat16)
        consts["be1b"] = np.tile(inp["be1"].reshape(1, D), (P, 1)).astype(ml_dtypes.bfloat16)
    if apply_g2:
        consts["g2b"] = np.tile(inp["g2"].reshape(1, D), (P, 1)).astype(ml_dtypes.bfloat16)
        consts["be2b"] = np.tile(inp["be2"].reshape(1, D), (P, 1)).astype(ml_dtypes.bfloat16)
    consts = {k: np.ascontiguousarray(v) for k, v in consts.items()}
    in_maps = [dict(consts, x=np.ascontiguousarray(x[i])) for i in range(B_)]
    return nc, in_maps


def kernel(**inputs):
    nc, in_maps = _prepare(inputs)
    res = run_bass_kernel_spmd(nc, in_maps, core_ids=list(range(B_)))
    return np.stack([res.results[i]["out"] for i in range(B_)], axis=0)


# revision 16
# speedup vs baseline: 1.2209x; 1.1143x over previous
"""Autoformer-style EncoderLayer (series-decomp + single-head attention + FFN)
for Trainium2, data-parallel over batch across 8 NeuronCores.

Per core: one [L=2048, D=512] sequence.
  trend = AvgPool1d(x, k=25, pad=12, count_include_pad=True)
  s     = x - trend                        (banded matmul: S = B @ x, B = I - A)
  Q,K,V = s@wq+bq, s@wk, s@wv              (bk cancels in softmax; bv folds into bo)
  attn  = softmax(Q K^T / sqrt(D))         (computed transposed: scores^T[m,l])
  h     = LN1(s + attn@V@wo + bo')         (bo' = bo + bv@wo)
  out   = LN2(h + relu(h@w1+bb1)@w2+bb2) + trend

Matmul operand dtypes: banded seasonal in f32r; scores (Q.K), attn@V and the
softmax denominator in fp8-e4m3 with DoubleRow perf mode (256-deep contraction
per pass -> half the MM count); everything else in bf16. PSUM accumulation and
LayerNorm statistics stay f32. exp() carries a -ln(8) bias so u fits fp8's
+-240 range; softmax normalization cancels the factor.

Scheduling notes:
- Everything is SBUF-resident (s/trend/h/Q^T/K^T/V/u); no DRAM spills.
- LayerNorm 1/sqrt(var) runs on the vector engine (bit-hack + 2 Newton steps)
  because scalar-engine Sqrt<->Exp alternation forces a 1.3us ACT_TABLE_LOAD
  per switch, stalling the Exp drains that pace the score chains.
- FFN work for an earlier block is emitted inside the next attention block
  (between the score chains and AV) so the PE has work while the scalar
  engine catches up on Exps; the PE never sees a phase boundary (keeps the
  HAM throttle at 8/8).
- Transposes write 4 chunks into one PSUM bank and drain with a single copy,
  alternating scalar/vector.
- The last block uses per-chunk LN2 so the exposed tail after the final
  matmul is one chunk's epilogue, not four.
"""
import math
import numpy as np
import ml_dtypes
from contextlib import ExitStack

import concourse.bass as bass
import concourse.mybir as mybir
import concourse.tile as tile
from concourse import bacc
from concourse.bass_utils import run_bass_kernel_spmd

P = 128
B_, L, D = 8, 2048, 512
KPOOL, PAD = 25, 12
EPS = 1e-5
SCALE = 1.0 / math.sqrt(D)
LOG8 = math.log(8.0)
NLC = L // P          # 16 l-chunks of 128
NB = L // 512         # 4  l-blocks of 512
ND = D // P           # 4  d-chunks of 128
USE_FP8 = True
MAGIC = 0x5F3759DF

f32 = mybir.dt.float32
f32r = mybir.dt.float32r
i32 = mybir.dt.int32
bf16 = mybir.dt.bfloat16
fp8 = mybir.dt.float8e4
AF = mybir.ActivationFunctionType
ALU = mybir.AluOpType
DRM = mybir.MatmulPerfMode.DoubleRow

_CACHE = {}


def _band_blocks():
    i = np.arange(P)[:, None]
    j = np.arange(P)[None, :]
    a = (np.abs(i - j) <= PAD).astype(np.float32) / KPOOL
    bdiag = np.eye(P, dtype=np.float32) - a
    bup = -((i - j) >= (P - PAD)).astype(np.float32) / KPOOL   # rows chunk c-1, cols chunk c
    bdown = bup.T.copy()                                       # rows chunk c+1, cols chunk c
    return bdiag, bup, bdown


def _make_ln_stats(nc, small, t_magic, t_sh1):
    """(sum, ssq)[P,n] -> (istd, nmi, negm), all on the vector engine.

    istd = rsqrt(var+eps) via the int bit-hack + 2 Newton iterations --
    avoids scalar-engine Sqrt, whose LUT alternates with Exp's and costs a
    1.3us ACT_TABLE_LOAD per switch.
    """
    def ln_stats(t_sum, t_ssq, n, tagp):
        t_mean = small.tile([P, n], f32, tag=tagp + "m", name="t_mean")
        nc.vector.tensor_scalar_mul(t_mean[:], t_sum[:], 1.0 / D)
        t_m2 = small.tile([P, n], f32, tag=tagp + "m2", name="t_m2")
        nc.vector.tensor_tensor(t_m2[:], t_mean[:], t_mean[:], ALU.mult)
        t_vpe = small.tile([P, n], f32, tag=tagp + "v", name="t_vpe")
        nc.vector.scalar_tensor_tensor(t_vpe[:], t_ssq[:], 1.0 / D, t_m2[:],
                                       op0=ALU.mult, op1=ALU.subtract)
        nc.vector.tensor_scalar_add(t_vpe[:], t_vpe[:], EPS)
        # y0 bits = magic - bits(v)/2, all through value converts (the DVE
        # int ALU shift/subtract path mis-executes on hw)
        t_y = small.tile([P, n], f32, tag=tagp + "y", name="t_y")
        t_b = small.tile([P, n], f32, tag=tagp + "b", name="t_b")
        nc.vector.tensor_copy(t_b[:], t_vpe.bitcast(i32)[:])
        nc.vector.tensor_scalar(t_b[:], t_b[:], -0.5, float(MAGIC),
                                op0=ALU.mult, op1=ALU.add)
        nc.vector.tensor_copy(t_y.bitcast(i32)[:], t_b[:])
        # Newton x2: y = y * (1.5 + (-0.5*v)*y^2)
        t_a = small.tile([P, n], f32, tag=tagp + "a", name="t_a")
        nc.vector.tensor_scalar_mul(t_a[:], t_vpe[:], -0.5)
        for _ in range(2):
            nc.vector.tensor_tensor(t_b[:], t_y[:], t_y[:], ALU.mult)
            nc.vector.tensor_tensor(t_b[:], t_b[:], t_a[:], ALU.mult)
            nc.vector.scalar_tensor_tensor(t_y[:], t_b[:], 1.5, t_y[:],
                                           op0=ALU.add, op1=ALU.mult)
        t_nmi = small.tile([P, n], f32, tag=tagp + "n", name="t_nmi")
        nc.vector.scalar_tensor_tensor(t_nmi[:], t_mean[:], -1.0, t_y[:],
                                       op0=ALU.mult, op1=ALU.mult)
        t_negm = small.tile([P, n], f32, tag=tagp + "g", name="t_negm")
        nc.vector.tensor_scalar_mul(t_negm[:], t_mean[:], -1.0)
        return t_y, t_nmi, t_negm
    return ln_stats


def _build(apply_g1, apply_g2):
    nc = bacc.Bacc("TRN2", target_bir_lowering=False, debug=False)

    def din(name, shape, dt):
        return nc.dram_tensor(name, list(shape), dt, kind="ExternalInput").ap()

    x = din("x", (L, D), f32)
    ws = {n: din(n, (D, D), bf16) for n in ["wq", "wk", "wv", "wo", "w1", "w2"]}
    cpk = din("cpk", (P, 416), f32)
    cbf = din("cbf", (P, 129), bf16)
    rbf = din("rbf", (1, 1154), bf16)
    gb = {}
    if apply_g1:
        gb["g1b"] = din("g1b", (P, D), bf16)
        gb["be1b"] = din("be1b", (P, D), bf16)
    if apply_g2:
        gb["g2b"] = din("g2b", (P, D), bf16)
        gb["be2b"] = din("be2b", (P, D), bf16)

    out = nc.dram_tensor("out", [L, D], f32, kind="ExternalOutput").ap()
    out_c = out.rearrange("(l p) d -> l p d", p=P)

    adt = fp8 if USE_FP8 else bf16   # attention operand dtype (q/k/v/u)

    with tile.TileContext(nc) as tc, ExitStack() as ctx:
        misc = ctx.enter_context(tc.tile_pool(name="misc", bufs=1))
        small = ctx.enter_context(tc.tile_pool(name="small", bufs=4))
        ps_mm = ctx.enter_context(tc.tile_pool(name="ps_mm", bufs=4, space="PSUM"))
        ps_tr = ctx.enter_context(tc.tile_pool(name="ps_tr", bufs=1, space="PSUM"))
        ps_den = ctx.enter_context(tc.tile_pool(name="ps_den", bufs=1, space="PSUM"))

        # ---- constants ----
        t_cpk = misc.tile([P, 416], f32r, name="t_cpk")
        nc.sync.dma_start(t_cpk[:], cpk.bitcast(f32r))
        t_cbf = misc.tile([P, 129], bf16, name="t_cbf")
        nc.sync.dma_start(t_cbf[:], cbf)
        t_bd = t_cpk[:, 0:128]
        t_bu = t_cpk[:, 128:256]
        t_bn = t_cpk[:, 256:384]
        t_eps = t_cpk[:, 384:385].bitcast(f32)
        t_bq = t_cpk[:, 385:389].bitcast(f32)
        t_bb1 = t_cpk[:, 389:393].bitcast(f32)
        t_o2 = t_cpk[0:1, 393:395]                    # f32r ones [1,2]
        t_nl8 = t_cpk[:, 395:396].bitcast(f32)        # -ln(8)
        t_magic = t_cpk[:, 396:400].bitcast(i32)      # rsqrt magic int x4
        t_sh1 = t_cpk[:, 400:401].bitcast(i32)        # int 1 (shift amount)
        t_id = t_cbf[:, 0:128]                        # bf16 identity
        t_ocol = t_cbf[:, 128:129]                    # bf16 ones column
        if USE_FP8:
            t_ones8 = misc.tile([P, 2, 16], fp8, name="t_ones8")
            nc.any.memset(t_ones8[:], 1.0)
        t_gb = {}
        for n in gb:
            t_gb[n] = misc.tile([P, D], bf16, name=f"t_{n}")
            nc.sync.dma_start(t_gb[n][:], gb[n][:])

        # ---- SBUF residents ----
        s_res = misc.tile([P, NLC, D], bf16, name="s_res")
        tr_res = misc.tile([P, NLC, D], bf16, name="tr_res")
        h_res = misc.tile([P, NLC, D], bf16, name="h_res")
        t_sum1a = misc.tile([P, NLC], f32, name="t_sum1a")
        t_ssq1a = misc.tile([P, NLC], f32, name="t_ssq1a")

        ln_stats = _make_ln_stats(nc, small, t_magic, t_sh1)

        wop = ctx.enter_context(tc.tile_pool(name="wop", bufs=1))
        t_wo = wop.tile([P, ND, D], bf16, name="t_wo")
        t_w1 = wop.tile([P, ND, D], bf16, name="t_w1")
        t_w2 = wop.tile([P, ND, D], bf16, name="t_w2")
        t_rbf = misc.tile([1, 1154], bf16, name="t_rbf")
        r_bo2 = t_rbf[:, 0:512]
        r_bb2 = t_rbf[:, 512:1024]
        r_ones = t_rbf[:, 1024:1152]

        # attention operand tensors
        abp = ctx.enter_context(tc.tile_pool(name="abp", bufs=1))
        t_qt = abp.tile([P, ND, L], adt, name="t_qt")
        t_kt = abp.tile([P, ND, L], adt, name="t_kt")
        t_v = abp.tile([P, NLC, D], adt, name="t_v")
        t_u = abp.tile([P, NLC, 512], adt, name="t_u")

        # ---- phase A: x -> s, trend, S^T, Q^T, K^T, V ----
        es_a = ExitStack()
        apool = es_a.enter_context(tc.tile_pool(name="apool", bufs=1))
        xwin = es_a.enter_context(tc.tile_pool(name="xwin", bufs=16))
        wqkv = es_a.enter_context(tc.tile_pool(name="wqkv", bufs=1))
        stb = apool.tile([P, ND, L], bf16, name="stb")
        x_cview = x.rearrange("(l p) d -> p l d", p=P)
        x_ch = [xwin.tile([P, D], f32r, tag="xw", name=f"xw{j}")
                for j in range(NLC)]
        t_w = {n: wqkv.tile([P, ND, D], bf16, name=f"t_w_{n}")
               for n in ["wq", "wk", "wv"]}

        def dx(j):
            nc.sync.dma_start(x_ch[j][:], x_cview[:, j, :].bitcast(f32r))

        def dw(t, n):
            # weights ride the gpsimd DMA queue: keeps the sync queue's
            # dynamic-descriptor count low (overflowing its scratch region
            # corrupts SBUF tiles) and lets weights stream in parallel with x
            nc.gpsimd.dma_start(t[:], ws[n].rearrange("(k p) n -> p k n", p=P))

        for j in range(NLC):
            dx(j)
        dw(t_w["wq"], "wq")
        dw(t_w["wk"], "wk")
        dw(t_w["wv"], "wv")
        nc.gpsimd.dma_start(t_rbf[:], rbf)
        dw(t_wo, "wo")
        dw(t_w1, "w1")
        dw(t_w2, "w2")

        for lb in range(NB):
            # banded seasonal + trend for this block
            for c in range(4):
                lc = lb * 4 + c
                pss = ps_mm.tile([P, D], f32, tag="mm", name="pss")
                nbrs = [(lc - 1, t_bu), (lc, t_bd), (lc + 1, t_bn)]
                nbrs = [(j, t) for j, t in nbrs if 0 <= j < NLC]
                for i, (j, tb) in enumerate(nbrs):
                    nc.tensor.matmul(pss[:], tb[:], x_ch[j][:],
                                     start=(i == 0), stop=(i == len(nbrs) - 1))
                nc.scalar.copy(s_res[:, lc, :], pss[:])
                nc.gpsimd.tensor_tensor(tr_res[:, lc, :],
                                        x_ch[lc][:].bitcast(f32),
                                        s_res[:, lc, :], ALU.subtract)
            # S^T: 4 transposes per chunk into one PSUM bank, one batched drain
            for c in range(4):
                lc = lb * 4 + c
                ptt = ps_tr.tile([P, ND, P], f32, tag="pt", bufs=2, name="ptt")
                for dc in range(ND):
                    nc.tensor.matmul(ptt[:, dc, :], s_res[:, lc, bass.ts(dc, P)],
                                     t_id[:], start=True, stop=True)
                if lc % 2 == 0:
                    nc.scalar.copy(stb[:, :, bass.ts(lc, P)], ptt[:])
                else:
                    nc.vector.tensor_copy(stb[:, :, bass.ts(lc, P)], ptt[:])
            for tdst, wname, has_b in [(t_qt, "wq", True), (t_kt, "wk", False)]:
                for dc in range(ND):
                    pq = ps_mm.tile([P, 512], f32, tag="mm", name="pq")
                    for k in range(ND):
                        nc.tensor.matmul(pq[:], t_w[wname][:, k, bass.ts(dc, P)],
                                         stb[:, k, bass.ts(lb, 512)],
                                         start=(k == 0), stop=(k == ND - 1))
                    if has_b:
                        nc.scalar.activation(tdst[:, dc, bass.ts(lb, 512)], pq[:],
                                             AF.Identity, bias=t_bq[:, dc:dc + 1])
                    else:
                        nc.scalar.copy(tdst[:, dc, bass.ts(lb, 512)], pq[:])
            for c in range(4):
                lc = lb * 4 + c
                pv = ps_mm.tile([P, D], f32, tag="mm", name="pv")
                for k in range(ND):
                    nc.tensor.matmul(pv[:], stb[:, k, bass.ts(lc, P)],
                                     t_w["wv"][:, k, :],
                                     start=(k == 0), stop=(k == ND - 1))
                nc.scalar.copy(t_v[:, lc, :], pv[:])
        es_a.close()   # frees stb, x window, wq/wk/wv

        # ---- phases B & C, interleaved ----
        bcp = ctx.enter_context(tc.tile_pool(name="bcp", bufs=1))
        cpool = ctx.enter_context(tc.tile_pool(name="cpool", bufs=1))
        t_ht = cpool.tile([P, ND, L], bf16, name="t_ht")
        t_rt = cpool.tile([P, ND, L], bf16, name="t_rt")
        rs_slabs = {}

        def emit_scores(lb):
            # scores^T (u[m, l]) with in-loop denominator accumulation
            pden = ps_den.tile([1, 512], f32, tag="den", name="pden")
            for mc in range(NLC):
                psc = ps_mm.tile([P, 512], f32, tag="mm", name="psc")
                if USE_FP8:
                    for k2 in (0, 2):
                        nc.tensor.matmul(psc[:], t_kt[:, k2:k2 + 2, bass.ts(mc, P)],
                                         t_qt[:, k2:k2 + 2, bass.ts(lb, 512)],
                                         start=(k2 == 0), stop=(k2 == 2),
                                         perf_mode=DRM)
                    nc.scalar.activation(t_u[:, mc, :], psc[:], AF.Exp,
                                         scale=SCALE, bias=t_nl8[:])
                    if mc % 2 == 1:
                        nc.tensor.matmul(pden[:], t_ones8[:, :, 0:1],
                                         t_u[:, mc - 1:mc + 1, :],
                                         start=(mc == 1), stop=(mc == NLC - 1),
                                         perf_mode=DRM)
                else:
                    for k in range(ND):
                        nc.tensor.matmul(psc[:], t_kt[:, k, bass.ts(mc, P)],
                                         t_qt[:, k, bass.ts(lb, 512)],
                                         start=(k == 0), stop=(k == ND - 1))
                    nc.scalar.activation(t_u[:, mc, :], psc[:], AF.Exp,
                                         scale=SCALE)
                    nc.tensor.matmul(pden[:], t_ocol[:], t_u[:, mc, :],
                                     start=(mc == 0), stop=(mc == NLC - 1))
            return pden

        def emit_attn_out(lb, pden):
            den_bf = small.tile([1, 512], bf16, tag="denb", name="den_bf")
            nc.scalar.copy(den_bf[:], pden[:])
            den_f = small.tile([1, 512], f32r, tag="denf", name="den_f")
            nc.scalar.copy(den_f[:], pden[:])
            prc = ps_tr.tile([P, 4, 2], f32, tag="rec", name="prc")
            for c in range(4):
                nc.tensor.matmul(prc[:, c, :], den_f[:, bass.ts(c, P)],
                                 t_o2[:], start=True, stop=True)
            t_rec = small.tile([P, 4], f32, tag="recs", name="t_rec")
            nc.vector.reciprocal(t_rec[:], prc[:, :, 0])

            # attn @ V  (transposed: avt[d, l])
            t_avt = bcp.tile([P, ND, 512], bf16, tag="avt", bufs=2, name="t_avt")
            for dc in range(ND):
                pav = ps_mm.tile([P, 512], f32, tag="mm", name="pav")
                if USE_FP8:
                    for m2 in range(0, NLC, 2):
                        nc.tensor.matmul(pav[:], t_v[:, m2:m2 + 2, bass.ts(dc, P)],
                                         t_u[:, m2:m2 + 2, :],
                                         start=(m2 == 0), stop=(m2 == NLC - 2),
                                         perf_mode=DRM)
                else:
                    for mc in range(NLC):
                        nc.tensor.matmul(pav[:], t_v[:, mc, bass.ts(dc, P)],
                                         t_u[:, mc, :],
                                         start=(mc == 0), stop=(mc == NLC - 1))
                nc.vector.tensor_copy(t_avt[:, dc, :], pav[:])
            return den_bf, t_rec, t_avt

        def emit_wo(lb, den_bf, t_rec, t_avt):
            # wo projection back to natural [l, d] + residual + LN1 stats
            rs_slab = bcp.tile([P, 4, D], f32, tag="rs", bufs=2, name="rs_slab")
            for c in range(4):
                lc = lb * 4 + c
                pwo = ps_mm.tile([P, D], f32, tag="mm", name="pwo")
                for k in range(ND):
                    nc.tensor.matmul(pwo[:], t_avt[:, k, bass.ts(c, P)],
                                     t_wo[:, k, :],
                                     start=(k == 0), stop=False)
                nc.tensor.matmul(pwo[:], den_bf[:, bass.ts(c, P)],
                                 r_bo2[:], start=False, stop=True)
                nc.vector.scalar_tensor_tensor(
                    rs_slab[:, c, :], pwo[:], t_rec[:, c:c + 1],
                    s_res[:, lc, :],
                    op0=ALU.mult, op1=ALU.add,
                    accum_out=t_sum1a[:, lc:lc + 1])
                t_scr = bcp.tile([P, D], f32, tag="sqscr", bufs=2, name="t_scr")
                nc.vector.scalar_tensor_tensor(
                    t_scr[:], rs_slab[:, c, :], 1.0, rs_slab[:, c, :],
                    op0=ALU.mult, op1=ALU.mult,
                    accum_out=t_ssq1a[:, lc:lc + 1])
            rs_slabs[lb] = rs_slab

        def emit_N(lb):
            # LN1 stats + normalize -> h (bf16), all vector
            t_istd4, t_nmi4, t_negm4 = ln_stats(
                t_sum1a[:, lb * 4:lb * 4 + 4],
                t_ssq1a[:, lb * 4:lb * 4 + 4], 4, "l1")
            for c in range(4):
                lc = lb * 4 + c
                nc.vector.tensor_scalar(h_res[:, lc, :], rs_slabs[lb][:, c, :],
                                        t_negm4[:, c:c + 1], t_istd4[:, c:c + 1],
                                        op0=ALU.add, op1=ALU.mult)
                if apply_g1:
                    nc.vector.tensor_tensor(h_res[:, lc, :], h_res[:, lc, :],
                                            t_gb["g1b"][:], ALU.mult)
                    nc.vector.tensor_tensor(h_res[:, lc, :], h_res[:, lc, :],
                                            t_gb["be1b"][:], ALU.add)

        def emit_ff1(lb):
            # h^T then ff1+relu
            for c in range(4):
                lc = lb * 4 + c
                pht = ps_tr.tile([P, ND, P], f32, tag="pt", bufs=2, name="pht")
                for dc in range(ND):
                    nc.tensor.matmul(pht[:, dc, :], h_res[:, lc, bass.ts(dc, P)],
                                     t_id[:], start=True, stop=True)
                if lc % 2 == 0:
                    nc.scalar.copy(t_ht[:, :, bass.ts(lc, P)], pht[:])
                else:
                    nc.vector.tensor_copy(t_ht[:, :, bass.ts(lc, P)], pht[:])
            for dc in range(ND):
                pf = ps_mm.tile([P, 512], f32, tag="mm", name="pf")
                for k in range(ND):
                    nc.tensor.matmul(pf[:], t_w1[:, k, bass.ts(dc, P)],
                                     t_ht[:, k, bass.ts(lb, 512)],
                                     start=(k == 0), stop=(k == ND - 1))
                nc.scalar.activation(t_rt[:, dc, bass.ts(lb, 512)], pf[:],
                                     AF.Relu, bias=t_bb1[:, dc:dc + 1])

        def emit_ff2(lb, last=False):
            # ff2 + LN2 + trend + out. last=True: per-chunk LN2 to cut the tail
            t_sum2b = small.tile([P, 4], f32, tag="sum2b", name="t_sum2b")
            t_ssq2b = small.tile([P, 4], f32, tag="ssq2b", name="t_ssq2b")
            res_list = []

            def finish(c, t_res, istd, nmi, negm):
                lc = lb * 4 + c
                t_h2 = cpool.tile([P, D], f32, tag="h2out", bufs=4, name="t_h2")
                if c % 2 == 1:
                    nc.scalar.activation(t_h2[:], t_res[:], AF.Identity,
                                         scale=istd, bias=nmi)
                else:
                    nc.vector.tensor_scalar(t_h2[:], t_res[:], negm, istd,
                                            op0=ALU.add, op1=ALU.mult)
                if apply_g2:
                    nc.vector.tensor_tensor(t_h2[:], t_h2[:],
                                            t_gb["g2b"][:], ALU.mult)
                    nc.vector.tensor_tensor(t_h2[:], t_h2[:],
                                            t_gb["be2b"][:], ALU.add)
                t_out = cpool.tile([P, D], f32, tag="outst", bufs=4, name="t_out")
                eng = nc.gpsimd if c % 2 == 0 else nc.vector
                eng.tensor_tensor(t_out[:], t_h2[:], tr_res[:, lc, :], ALU.add)
                nc.gpsimd.dma_start(out_c[lc], t_out[:])

            for c in range(4):
                lc = lb * 4 + c
                pf2 = ps_mm.tile([P, D], f32, tag="mm", name="pf2")
                for k in range(ND):
                    nc.tensor.matmul(pf2[:], t_rt[:, k, bass.ts(lc, P)],
                                     t_w2[:, k, :],
                                     start=(k == 0), stop=False)
                nc.tensor.matmul(pf2[:], r_ones[:], r_bb2[:],
                                 start=False, stop=True)
                t_res = cpool.tile([P, D], f32, tag="res2", bufs=5, name="t_res2")
                nc.vector.scalar_tensor_tensor(
                    t_res[:], pf2[:], 1.0, h_res[:, lc, :],
                    op0=ALU.mult, op1=ALU.add,
                    accum_out=t_sum2b[:, c:c + 1])
                t_scr = cpool.tile([P, D], f32, tag="sqscr2", bufs=2,
                                   name="t_scr2")
                nc.scalar.activation(t_scr[:], t_res[:], AF.Square,
                                     accum_out=t_ssq2b[:, c:c + 1])
                res_list.append(t_res)
                if last:
                    istd1, nmi1, negm1 = ln_stats(t_sum2b[:, c:c + 1],
                                                  t_ssq2b[:, c:c + 1], 1, "l2")
                    finish(c, t_res, istd1[:, 0:1], nmi1[:, 0:1],
                           negm1[:, 0:1])
            if not last:
                t_istd4, t_nmi4, t_negm4 = ln_stats(t_sum2b, t_ssq2b, 4, "l2")
                for c in range(4):
                    finish(c, res_list[c], t_istd4[:, c:c + 1],
                           t_nmi4[:, c:c + 1], t_negm4[:, c:c + 1])

        # segment interleave: B(lb) hides C(lb-2) work behind Exp latency
        def emit_seg(lb, prev):
            pden = emit_scores(lb)
            if prev is not None:
                emit_ff1(prev)
            den_bf, t_rec, t_avt = emit_attn_out(lb, pden)
            if prev is not None:
                emit_ff2(prev)
            emit_wo(lb, den_bf, t_rec, t_avt)
            emit_N(lb)

        INTERLEAVE = False
        if INTERLEAVE:
            emit_seg(0, None)
            emit_seg(1, None)
            emit_seg(2, 0)
            emit_seg(3, 1)
            emit_ff1(2)
            emit_ff2(2)
            emit_ff1(3)
            emit_ff2(3, last=True)
        else:
            for lb in range(NB):
                emit_seg(lb, None)
            for lb in range(NB):
                emit_ff1(lb)
                emit_ff2(lb, last=(lb == NB - 1))

    nc.compile()
    return nc


def _consts(inp):
    bdiag, bup, bdown = _band_blocks()
    cpk = np.zeros((P, 416), np.float32)
    cpk[:, 0:128] = bdiag
    cpk[:, 128:256] = bup
    cpk[:, 256:384] = bdown
    cpk[:, 384] = EPS
    cpk[:, 385:389] = inp["bq"].reshape(ND, P).T
    cpk[:, 389:393] = inp["bb1"].reshape(ND, P).T
    cpk[:, 393:395] = 1.0
    cpk[:, 395] = -LOG8
    cpk[:, 396:400] = np.full((P, 4), MAGIC, np.int32).view(np.float32)
    cpk[:, 400] = np.full(P, 1, np.int32).view(np.float32)
    cbf = np.zeros((P, 129), ml_dtypes.bfloat16)
    cbf[:, 0:128] = np.eye(P, dtype=np.float32)
    cbf[:, 128] = 1.0
    wo_b = inp["wo"].astype(ml_dtypes.bfloat16).astype(np.float32)
    bo2 = inp["bo"] + inp["bv"].astype(ml_dtypes.bfloat16).astype(np.float32) @ wo_b
    rbf = np.zeros((1, 1154), ml_dtypes.bfloat16)
    rbf[0, 0:512] = bo2
    rbf[0, 512:1024] = inp["bb2"]
    rbf[0, 1024:1152] = 1.0
    consts = {"cpk": cpk, "cbf": cbf, "rbf": rbf}
    for n in ["wq", "wk", "wv", "wo", "w1", "w2"]:
        consts[n] = inp[n].astype(ml_dtypes.bfloat16)
    return consts


def _prepare(inputs):
    inp = {k: np.ascontiguousarray(np.asarray(v, dtype=np.float32))
           for k, v in inputs.items()}
    x = inp["x"]                      # [8, 2048, 512]
    assert x.shape == (B_, L, D)

    apply_g1 = not (np.allclose(inp["g1"], 1.0) and np.allclose(inp["be1"], 0.0))
    apply_g2 = not (np.allclose(inp["g2"], 1.0) and np.allclose(inp["be2"], 0.0))

    key = (apply_g1, apply_g2)
    if key not in _CACHE:
        _CACHE[key] = _build(apply_g1, apply_g2)
    nc = _CACHE[key]

    consts = _consts(inp)
    if apply_g1:
        consts["g1b"] = np.tile(inp["g1"].reshape(1, D), (P, 1)).astype(ml_dtypes.bfloat16)
        consts["be1b"] = np.tile(inp["be1"].reshape(1, D), (P, 1)).astype(ml_dtypes.bfloat16)
    if apply_g2:
        consts["g2b"] = np.tile(inp["g2"].reshape(1, D), (P, 1)).astype(ml_dtypes.bfloat16)
        consts["be2b"] = np.tile(inp["be2"].reshape(1, D), (P, 1)).astype(ml_dtypes.bfloat16)
    consts = {k: np.ascontiguousarray(v) for k, v in consts.items()}
    in_maps = [dict(consts, x=np.ascontiguousarray(x[i])) for i in range(B_)]
    return nc, in_maps


def kernel(**inputs):
    nc, in_maps = _prepare(inputs)
    res = run_bass_kernel_spmd(nc, in_maps, core_ids=list(range(B_)))
    return np.stack([res.results[i]["out"] for i in range(B_)], axis=0)
